# revision 2
# baseline (speedup 1.0000x reference)
"""MoE (brute-force reference) kernel for 8 TRN2 NeuronCores.

Strategy: expert-parallel. Host routes token-slots by gate_idx to their
expert, dedups the top-k pairs, packs each expert's slots to capacity C,
and pre-permutes every tensor into SBUF partition-major layout so each
DMA is one contiguous run per partition (128 descriptors, line-rate).
Each core owns 2 experts and computes
  hT[m] = gelu(sum_k w1T[k,m].T @ xT[k] + b1)   then
  yT[m] = sum_k w2T[k,m].T @ hT[k]
All matmul operands are fp16 (same PE rate as bf16, ~8x the accuracy);
accumulation is fp32 in PSUM. b1 is applied on-device (per-partition
bias fused into the gelu activation); b2 and the gate_score combine
happen on host in exact fp32. Capacity overflow (>C distinct tokens on
one expert) is computed exactly on host — rare and tiny.

Head: xt + the first w1 slabs issue first on the sync HWDGE ring so the
first real matmul starts ~3.5us after engine start; a tunable dummy-
matmul warm-up keeps the PE HAM clock busy until then so the stream
runs at 2.4GHz from the first real matmul. w2 issues on the scalar
HWDGE ring to halve the serialized descriptor-issue chain.
"""

import numpy as np

import concourse.bacc as bacc
import concourse.mybir as mybir
from concourse import tile
from concourse.bass_utils import run_bass_kernel_spmd

E, D, H, TOPK, T = 16, 1024, 2048, 2, 2048
NCORES = 8
EPC = E // NCORES  # experts per core
C = 256            # per-expert token capacity after top-k dedup
                   # (overflow handled exactly on host)
KD, KH, MD = D // 128, H // 128, D // 128  # 8, 16, 8
HH = H // 2        # w1 column half (A = m-tiles 0..7, B = 8..15)
WARM = 38          # PE warm-up matmuls (N=128, cold ~107ns each)

_F16 = np.float16
_CACHE: dict = {}


def _build(reps: int = 1):
    dt = mybir.dt.float16
    f32 = mybir.dt.float32
    nc = bacc.Bacc("TRN2", target_bir_lowering=False, debug=False,
                   num_devices=NCORES)
    # All DRAM tensors are partition-major: [...,128, free] so a DMA of
    # any [128, n] slice is one contiguous run per partition.
    xt = nc.dram_tensor("xt", [EPC, 128, KD * C], dt, kind="ExternalInput")
    w1a = nc.dram_tensor("w1a", [EPC, 128, KD * HH], dt, kind="ExternalInput")
    w1b = nc.dram_tensor("w1b", [EPC, 128, KD * HH], dt, kind="ExternalInput")
    w2t = nc.dram_tensor("w2t", [EPC, 128, KH * D], dt, kind="ExternalInput")
    b1 = nc.dram_tensor("b1", [EPC, 128, KH], f32, kind="ExternalInput")
    yt = nc.dram_tensor("yt", [EPC, 128, MD * C], dt, kind="ExternalOutput")

    gelu = mybir.ActivationFunctionType.Gelu_apprx_tanh
    MGRP = 8   # GEMM1 m-tiles per psum group (k-inner within a group)

    with tile.TileContext(nc) as tc:
        with (
            tc.tile_pool(name="xtp", bufs=2) as xtp,
            tc.tile_pool(name="w1p", bufs=1) as w1p,
            tc.tile_pool(name="w2p", bufs=1) as w2p,
            tc.tile_pool(name="htp", bufs=2) as htp,
            tc.tile_pool(name="yp", bufs=4) as yp,
            tc.tile_pool(name="bp", bufs=2) as bp,
            tc.tile_pool(name="ps", bufs=1, space="PSUM") as psp,
        ):
            # PE warm-up: tiny back-to-back matmuls while the first input
            # DMAs stream in, so the HAM clock gate is at 8/8 when the
            # real matmul stream starts.
            zt = bp.tile([128, 128], dt, name="warmz", tag="warmz")
            nc.any.memset(zt[:], 0.0)
            psw = psp.tile([128, 128], f32, name="psw", tag="ps7")
            for _ in range(WARM):
                nc.tensor.matmul(psw[:], zt[:], zt[:],
                                 start=True, stop=True)

            for r in range(reps):
                for e in range(EPC):
                    u = f"{r}_{e}"
                    first = (r == 0 and e == 0)
                    last = (r == reps - 1 and e == EPC - 1)

                    # --- input DMAs ------------------------------------
                    # First expert: xt + w1 A-slabs first on sync (HWDGE)
                    # so the stream can start ASAP; w2 on the scalar
                    # HWDGE ring. Second expert: merged chunks, xt/b1 on
                    # gpsimd (SWDGE) to keep HWDGE free for weights.
                    xte = xtp.tile([128, KD * C], dt, name=f"xt{u}", tag="xt")
                    x_eng = nc.sync if first else nc.gpsimd
                    x_eng.dma_start(out=xte[:], in_=xt.ap()[e])

                    def xtv(k):
                        return xte[:, k * C:(k + 1) * C]

                    b1s = bp.tile([128, KH], f32, name=f"b1s{u}", tag="b1s")
                    b_eng = nc.sync if first else nc.gpsimd
                    b_eng.dma_start(out=b1s[:], in_=b1.ap()[e])

                    # w1 A half: slab k = columns [k*HH, (k+1)*HH) of the
                    # packed A tensor. First expert streams per-slab so
                    # GEMM1 group 0 starts as soon as slab 0 lands.
                    w1av = w1a.ap()[e]
                    if first:
                        w1as = []
                        for k in range(KD):
                            tl = w1p.tile([128, HH], dt, name=f"w1a{u}_{k}",
                                          tag=f"w1aS{k}")
                            nc.sync.dma_start(
                                out=tl[:], in_=w1av[:, k * HH:(k + 1) * HH])
                            w1as.append(tl[:])
                    else:
                        w1as = []
                        for j in range(2):
                            tl = w1p.tile([128, 4 * HH], dt, name=f"w1a{u}_{j}",
                                          tag=f"w1aM{j}")
                            nc.sync.dma_start(
                                out=tl[:],
                                in_=w1av[:, j * 4 * HH:(j + 1) * 4 * HH])
                            for k in range(4):
                                w1as.append(tl[:, k * HH:(k + 1) * HH])

                    # w1 B half
                    w1bv = w1b.ap()[e]
                    w1bs = []
                    nb = 2 if first else 1
                    for j in range(nb):
                        w = KD // nb
                        tl = w1p.tile([128, w * HH], dt, name=f"w1b{u}_{j}",
                                      tag=f"w1b{'S' if first else 'M'}{j}")
                        nc.sync.dma_start(
                            out=tl[:], in_=w1bv[:, j * w * HH:(j + 1) * w * HH])
                        for k in range(w):
                            w1bs.append(tl[:, k * HH:(k + 1) * HH])

                    # w2: slab k (of KH) = columns [k*D, (k+1)*D). Issue
                    # on the scalar HWDGE ring.
                    w2v = w2t.ap()[e]
                    w2s = []
                    nw = 4 if first else 2
                    for j in range(nw):
                        w = KH // nw
                        tl = w2p.tile([128, w * D], dt, name=f"w2{u}_{j}",
                                      tag=f"w2{'S' if first else 'M'}{j}")
                        nc.scalar.dma_start(
                            out=tl[:], in_=w2v[:, j * w * D:(j + 1) * w * D])
                        for k in range(w):
                            w2s.append(tl[:, k * D:(k + 1) * D])

                    # --- GEMM1: hT[m] = gelu(sum_k w1[k][:,m].T @ xT[k] + b1)
                    hts = [htp.tile([128, C], dt, name=f"ht{u}_{m}",
                                    tag=f"ht{m}") for m in range(KH)]
                    for g in range(0, KH, MGRP):
                        w1h = w1as if g == 0 else w1bs
                        pss = [psp.tile([128, C], f32, name=f"ps1_{u}_{m}",
                                        tag=f"ps{m - g}")
                               for m in range(g, g + MGRP)]
                        for k in range(KD):
                            for i, m in enumerate(range(g, g + MGRP)):
                                mm = m - g
                                nc.tensor.matmul(
                                    pss[i][:],
                                    w1h[k][:, mm * 128:(mm + 1) * 128],
                                    xtv(k),
                                    start=(k == 0), stop=(k == KD - 1))
                        for i, m in enumerate(range(g, g + MGRP)):
                            nc.scalar.activation(
                                hts[m][:], pss[i][:], gelu,
                                bias=b1s[:, m:m + 1])

                    # --- GEMM2: yT[m] = sum_k w2s[k][:,m].T @ hts[k] ----
                    # k-inner per single m so evictions stream. Earlier
                    # experts merge 4 m-tiles per output DMA (gpsimd);
                    # the last expert's outputs go per-m on the idle sync
                    # HWDGE so the final transfer on the critical tail is
                    # small.
                    ygrp = 1 if last else 4
                    ytv = yt.ap()[e]
                    for g in range(0, MD, ygrp):
                        yo = yp.tile([128, ygrp * C], dt, name=f"y{u}_{g}",
                                     tag=f"y{'L' if last else ''}{g % 4}")
                        for i, m in enumerate(range(g, g + ygrp)):
                            ps = psp.tile([128, C], f32, name=f"ps2_{u}_{m}",
                                          tag=f"ps{m % MGRP}")
                            for k in range(KH):
                                nc.tensor.matmul(
                                    ps[:],
                                    w2s[k][:, m * 128:(m + 1) * 128],
                                    hts[k][:],
                                    start=(k == 0), stop=(k == KH - 1))
                            nc.vector.tensor_copy(
                                out=yo[:, i * C:(i + 1) * C], in_=ps[:])
                        y_eng = nc.sync if last else nc.gpsimd
                        y_eng.dma_start(
                            out=ytv[:, g * C:(g + ygrp) * C], in_=yo[:])
    nc.compile()
    return nc


def _get_nc(reps: int = 1):
    if reps not in _CACHE:
        _CACHE[reps] = _build(reps)
    return _CACHE[reps]


def _route(gate_idx, gate_score):
    """Dedup routing: tokens whose two top-k picks are the same expert are
    sent once with summed score. Returns per-expert (tokens, weights,
    overflow_tokens, overflow_weights)."""
    g = np.asarray(gate_idx).astype(np.int64)
    sc = np.asarray(gate_score, dtype=np.float32)
    out = []
    for e in range(E):
        m0, m1 = g[:, 0] == e, g[:, 1] == e
        toks = np.flatnonzero(m0 | m1)
        wts = (sc[:, 0] * m0 + sc[:, 1] * m1)[toks]
        out.append((toks[:C], wts[:C], toks[C:], wts[C:]))
    return out


def _pmajor(a, kt):
    """[rows=kt*128, cols] -> SBUF partition-major [128, kt*cols]."""
    cols = a.shape[1]
    return np.ascontiguousarray(
        a.reshape(kt, 128, cols).transpose(1, 0, 2).reshape(128, kt * cols))


def kernel(inp, gate_idx, gate_score, w1, b1, w2, b2):
    inp = np.asarray(inp, dtype=np.float32)
    gate_idx = np.asarray(gate_idx)
    gate_score = np.asarray(gate_score, dtype=np.float32)
    w1 = np.asarray(w1, dtype=np.float32)
    b1 = np.asarray(b1, dtype=np.float32)
    w2 = np.asarray(w2, dtype=np.float32)
    b2 = np.asarray(b2, dtype=np.float32)

    routes = _route(gate_idx, gate_score)

    # Host-side gather + transpose to fp16 partition-major device layout.
    xt_all = np.zeros((E, 128, KD * C), dtype=_F16)
    for e in range(E):
        toks = routes[e][0]
        if len(toks):
            xe = np.zeros((D, C), dtype=_F16)
            xe[:, :len(toks)] = inp[toks].T.astype(_F16)
            xt_all[e] = _pmajor(xe, KD)

    w1t = w1.transpose(0, 2, 1).astype(_F16)          # [E, D, H]
    w2tt = w2.transpose(0, 2, 1).astype(_F16)         # [E, H, D]
    w1a_all = np.stack([_pmajor(w1t[e][:, :HH], KD) for e in range(E)])
    w1b_all = np.stack([_pmajor(w1t[e][:, HH:], KD) for e in range(E)])
    w2_all = np.stack([_pmajor(w2tt[e], KH) for e in range(E)])
    b1_all = np.ascontiguousarray(
        b1.reshape(E, KH, 128).transpose(0, 2, 1))

    in_maps = []
    for c in range(NCORES):
        sl = slice(EPC * c, EPC * (c + 1))
        in_maps.append({
            "xt": xt_all[sl],
            "w1a": w1a_all[sl],
            "w1b": w1b_all[sl],
            "w2t": w2_all[sl],
            "b1": b1_all[sl],
        })

    nc = _get_nc()
    res = run_bass_kernel_spmd(nc, in_maps, list(range(NCORES)))

    # Host combine: weight each expert's output columns by the (summed)
    # gate score and accumulate per token; add the b2 term (folded out of
    # the device kernel). Tokens are unique within an expert, so the
    # fancy-indexed += is safe.
    out = np.einsum("tk,tkd->td", np.asarray(gate_score, dtype=np.float32),
                    b2[np.asarray(gate_idx).astype(np.int64)])
    out = np.ascontiguousarray(out, dtype=np.float32)
    for e in range(E):
        core, le = divmod(e, EPC)
        toks, wts, otoks, owts = routes[e]
        if len(toks):
            # yt [128, MD*C] partition-major -> [D, C]
            ytp = res.results[core]["yt"][le].reshape(128, MD, C)
            y = ytp.transpose(1, 0, 2).reshape(D, C)[:, :len(toks)]
            out[toks] += wts[:, None] * y.T.astype(np.float32)
        if len(otoks):  # exact host fallback for capacity overflow
            hh = inp[otoks] @ w1[e].T + b1[e]
            hh = 0.5 * hh * (1.0 + np.tanh(
                np.sqrt(2.0 / np.pi) * (hh + 0.044715 * hh ** 3)))
            out[otoks] += owts[:, None] * (hh @ w2[e].T)
    return out


# revision 5
# speedup vs baseline: 1.0929x; 1.0929x over previous
"""MoE (brute-force reference) kernel for 8 TRN2 NeuronCores.

Strategy: expert-parallel. Host routes token-slots by gate_idx to their
expert, dedups the top-k pairs, packs each expert's slots to capacity C,
and pre-permutes every tensor into SBUF partition-major layout so each
DMA is one contiguous run per partition (128 descriptors, line-rate).
Each core owns 2 experts and computes
  hT[m] = gelu(sum_k w1T[k,m].T @ xT[k] + b1)   then
  yT[m] = sum_k w2T[k,m].T @ hT[k]
All matmul operands are fp16 (same PE rate as bf16, ~8x the accuracy);
accumulation is fp32 in PSUM. b1 is applied on-device (per-partition
bias fused into the gelu activation); b2 and the gate_score combine
happen on host in exact fp32. Capacity overflow (>C distinct tokens on
one expert) is computed exactly on host — rare and tiny.

Head: xt + the first w1 slabs issue first on the sync HWDGE ring so the
first real matmul starts ~3.5us after engine start; a tunable dummy-
matmul warm-up keeps the PE HAM clock busy until then so the stream
runs at 2.4GHz from the first real matmul. w2 issues on the scalar
HWDGE ring to halve the serialized descriptor-issue chain.
"""

import numpy as np

import concourse.bacc as bacc
import concourse.mybir as mybir
from concourse import tile
from concourse.bass_utils import run_bass_kernel_spmd

E, D, H, TOPK, T = 16, 1024, 2048, 2, 2048
NCORES = 8
EPC = E // NCORES  # experts per core
C = 256            # per-expert token capacity after top-k dedup
                   # (overflow handled exactly on host)
KD, KH, MD = D // 128, H // 128, D // 128  # 8, 16, 8
HH = H // 2        # w1 column half (A = m-tiles 0..7, B = 8..15)
WARM = 38          # PE warm-up matmuls (N=128, cold ~107ns each)

_F16 = np.float16
_CACHE: dict = {}

# Build-time tunables (sim-sweepable).
OPTS = dict(
    warm=WARM,      # warm-up matmul count
    ygrp=4,         # mid-kernel y m-tiles per output DMA
    ygrp_last=1,    # last expert y m-tiles per output DMA
    w2_eng="sync",  # HWDGE ring for w2 ("scalar" serializes ACT table loads)
    y_eng="gpsimd",   # mid-kernel y DMA engine
)


def _build(reps: int = 1):
    dt = mybir.dt.float16
    f32 = mybir.dt.float32
    nc = bacc.Bacc("TRN2", target_bir_lowering=False, debug=False,
                   num_devices=NCORES)
    # All DRAM tensors are partition-major: [...,128, free] so a DMA of
    # any [128, n] slice is one contiguous run per partition.
    xt = nc.dram_tensor("xt", [EPC, 128, KD * C], dt, kind="ExternalInput")
    w1a = nc.dram_tensor("w1a", [EPC, 128, KD * HH], dt, kind="ExternalInput")
    w1b = nc.dram_tensor("w1b", [EPC, 128, KD * HH], dt, kind="ExternalInput")
    w2t = nc.dram_tensor("w2t", [EPC, 128, KH * D], dt, kind="ExternalInput")
    b1 = nc.dram_tensor("b1", [EPC, 128, KH], f32, kind="ExternalInput")
    yt = nc.dram_tensor("yt", [EPC, 128, MD * C], dt, kind="ExternalOutput")

    gelu = mybir.ActivationFunctionType.Gelu_apprx_tanh
    MGRP = 8   # GEMM1 m-tiles per psum group (k-inner within a group)

    with tile.TileContext(nc) as tc:
        with (
            tc.tile_pool(name="xtp", bufs=2) as xtp,
            tc.tile_pool(name="w1p", bufs=1) as w1p,
            tc.tile_pool(name="w2p", bufs=1) as w2p,
            tc.tile_pool(name="htp", bufs=2) as htp,
            tc.tile_pool(name="yp", bufs=4) as yp,
            tc.tile_pool(name="bp", bufs=2) as bp,
            tc.tile_pool(name="ps", bufs=1, space="PSUM") as psp,
        ):
            # PE warm-up: tiny back-to-back matmuls while the first input
            # DMAs stream in, so the HAM clock gate is at 8/8 when the
            # real matmul stream starts.
            zt = bp.tile([128, 128], dt, name="warmz", tag="warmz")
            nc.any.memset(zt[:], 0.0)
            psw = psp.tile([128, 128], f32, name="psw", tag="ps7")
            for _ in range(OPTS['warm']):
                nc.tensor.matmul(psw[:], zt[:], zt[:],
                                 start=True, stop=True)

            for r in range(reps):
                for e in range(EPC):
                    u = f"{r}_{e}"
                    first = (r == 0 and e == 0)
                    last = (r == reps - 1 and e == EPC - 1)

                    # --- input DMAs ------------------------------------
                    # First expert: xt + w1 A-slabs first on sync (HWDGE)
                    # so the stream can start ASAP; w2 on the scalar
                    # HWDGE ring. Second expert: merged chunks, xt/b1 on
                    # gpsimd (SWDGE) to keep HWDGE free for weights.
                    xte = xtp.tile([128, KD * C], dt, name=f"xt{u}", tag="xt")
                    x_eng = nc.sync if first else nc.gpsimd
                    x_eng.dma_start(out=xte[:], in_=xt.ap()[e])

                    def xtv(k):
                        return xte[:, k * C:(k + 1) * C]

                    b1s = bp.tile([128, KH], f32, name=f"b1s{u}", tag="b1s")
                    b_eng = nc.sync if first else nc.gpsimd
                    b_eng.dma_start(out=b1s[:], in_=b1.ap()[e])

                    # w1 A half: slab k = columns [k*HH, (k+1)*HH) of the
                    # packed A tensor. First expert streams per-slab so
                    # GEMM1 group 0 starts as soon as slab 0 lands.
                    w1av = w1a.ap()[e]
                    if first:
                        w1as = []
                        for k in range(KD):
                            tl = w1p.tile([128, HH], dt, name=f"w1a{u}_{k}",
                                          tag=f"w1aS{k}")
                            nc.sync.dma_start(
                                out=tl[:], in_=w1av[:, k * HH:(k + 1) * HH])
                            w1as.append(tl[:])
                    else:
                        w1as = []
                        for j in range(2):
                            tl = w1p.tile([128, 4 * HH], dt, name=f"w1a{u}_{j}",
                                          tag=f"w1aM{j}")
                            nc.sync.dma_start(
                                out=tl[:],
                                in_=w1av[:, j * 4 * HH:(j + 1) * 4 * HH])
                            for k in range(4):
                                w1as.append(tl[:, k * HH:(k + 1) * HH])

                    # w1 B half
                    w1bv = w1b.ap()[e]
                    w1bs = []
                    nb = 2 if first else 1
                    for j in range(nb):
                        w = KD // nb
                        tl = w1p.tile([128, w * HH], dt, name=f"w1b{u}_{j}",
                                      tag=f"w1b{'S' if first else 'M'}{j}")
                        nc.sync.dma_start(
                            out=tl[:], in_=w1bv[:, j * w * HH:(j + 1) * w * HH])
                        for k in range(w):
                            w1bs.append(tl[:, k * HH:(k + 1) * HH])

                    # w2: slab k (of KH) = columns [k*D, (k+1)*D). Issue
                    # on the scalar HWDGE ring.
                    w2v = w2t.ap()[e]
                    w2s = []
                    nw = 4 if first else 2
                    for j in range(nw):
                        w = KH // nw
                        tl = w2p.tile([128, w * D], dt, name=f"w2{u}_{j}",
                                      tag=f"w2{'S' if first else 'M'}{j}")
                        w2_eng = nc.scalar if OPTS['w2_eng'] == 'scalar' else nc.sync
                        w2_eng.dma_start(
                            out=tl[:], in_=w2v[:, j * w * D:(j + 1) * w * D])
                        for k in range(w):
                            w2s.append(tl[:, k * D:(k + 1) * D])

                    # --- GEMM1: hT[m] = gelu(sum_k w1[k][:,m].T @ xT[k] + b1)
                    hts = [htp.tile([128, C], dt, name=f"ht{u}_{m}",
                                    tag=f"ht{m}") for m in range(KH)]
                    for g in range(0, KH, MGRP):
                        w1h = w1as if g == 0 else w1bs
                        pss = [psp.tile([128, C], f32, name=f"ps1_{u}_{m}",
                                        tag=f"ps{m - g}")
                               for m in range(g, g + MGRP)]
                        for k in range(KD):
                            for i, m in enumerate(range(g, g + MGRP)):
                                mm = m - g
                                nc.tensor.matmul(
                                    pss[i][:],
                                    w1h[k][:, mm * 128:(mm + 1) * 128],
                                    xtv(k),
                                    start=(k == 0), stop=(k == KD - 1))
                        for i, m in enumerate(range(g, g + MGRP)):
                            nc.scalar.activation(
                                hts[m][:], pss[i][:], gelu,
                                bias=b1s[:, m:m + 1])

                    # --- GEMM2: yT[m] = sum_k w2s[k][:,m].T @ hts[k] ----
                    # k-inner per single m so evictions stream. Earlier
                    # experts merge 4 m-tiles per output DMA (gpsimd);
                    # the last expert's outputs go per-m on the idle sync
                    # HWDGE so the final transfer on the critical tail is
                    # small.
                    ygrp = OPTS['ygrp_last'] if last else OPTS['ygrp']
                    ytv = yt.ap()[e]
                    for g in range(0, MD, ygrp):
                        yo = yp.tile([128, ygrp * C], dt, name=f"y{u}_{g}",
                                     tag=f"y{'L' if last else ''}{g % 4}")
                        for i, m in enumerate(range(g, g + ygrp)):
                            ps = psp.tile([128, C], f32, name=f"ps2_{u}_{m}",
                                          tag=f"ps{m % MGRP}")
                            for k in range(KH):
                                nc.tensor.matmul(
                                    ps[:],
                                    w2s[k][:, m * 128:(m + 1) * 128],
                                    hts[k][:],
                                    start=(k == 0), stop=(k == KH - 1))
                            nc.vector.tensor_copy(
                                out=yo[:, i * C:(i + 1) * C], in_=ps[:])
                        y_eng = nc.sync if last else (nc.gpsimd if OPTS['y_eng'] == 'gpsimd' else nc.sync)
                        y_eng.dma_start(
                            out=ytv[:, g * C:(g + ygrp) * C], in_=yo[:])
    nc.compile()
    return nc


def _get_nc(reps: int = 1):
    if reps not in _CACHE:
        _CACHE[reps] = _build(reps)
    return _CACHE[reps]


def _route(gate_idx, gate_score):
    """Dedup routing: tokens whose two top-k picks are the same expert are
    sent once with summed score. Returns per-expert (tokens, weights,
    overflow_tokens, overflow_weights)."""
    g = np.asarray(gate_idx).astype(np.int64)
    sc = np.asarray(gate_score, dtype=np.float32)
    out = []
    for e in range(E):
        m0, m1 = g[:, 0] == e, g[:, 1] == e
        toks = np.flatnonzero(m0 | m1)
        wts = (sc[:, 0] * m0 + sc[:, 1] * m1)[toks]
        out.append((toks[:C], wts[:C], toks[C:], wts[C:]))
    return out


def _pmajor(a, kt):
    """[rows=kt*128, cols] -> SBUF partition-major [128, kt*cols]."""
    cols = a.shape[1]
    return np.ascontiguousarray(
        a.reshape(kt, 128, cols).transpose(1, 0, 2).reshape(128, kt * cols))


def kernel(inp, gate_idx, gate_score, w1, b1, w2, b2):
    inp = np.asarray(inp, dtype=np.float32)
    gate_idx = np.asarray(gate_idx)
    gate_score = np.asarray(gate_score, dtype=np.float32)
    w1 = np.asarray(w1, dtype=np.float32)
    b1 = np.asarray(b1, dtype=np.float32)
    w2 = np.asarray(w2, dtype=np.float32)
    b2 = np.asarray(b2, dtype=np.float32)

    routes = _route(gate_idx, gate_score)

    # Host-side gather + transpose to fp16 partition-major device layout.
    xt_all = np.zeros((E, 128, KD * C), dtype=_F16)
    for e in range(E):
        toks = routes[e][0]
        if len(toks):
            xe = np.zeros((D, C), dtype=_F16)
            xe[:, :len(toks)] = inp[toks].T.astype(_F16)
            xt_all[e] = _pmajor(xe, KD)

    w1t = w1.transpose(0, 2, 1).astype(_F16)          # [E, D, H]
    w2tt = w2.transpose(0, 2, 1).astype(_F16)         # [E, H, D]
    w1a_all = np.stack([_pmajor(w1t[e][:, :HH], KD) for e in range(E)])
    w1b_all = np.stack([_pmajor(w1t[e][:, HH:], KD) for e in range(E)])
    w2_all = np.stack([_pmajor(w2tt[e], KH) for e in range(E)])
    b1_all = np.ascontiguousarray(
        b1.reshape(E, KH, 128).transpose(0, 2, 1))

    in_maps = []
    for c in range(NCORES):
        sl = slice(EPC * c, EPC * (c + 1))
        in_maps.append({
            "xt": xt_all[sl],
            "w1a": w1a_all[sl],
            "w1b": w1b_all[sl],
            "w2t": w2_all[sl],
            "b1": b1_all[sl],
        })

    nc = _get_nc()
    res = run_bass_kernel_spmd(nc, in_maps, list(range(NCORES)))

    # Host combine: weight each expert's output columns by the (summed)
    # gate score and accumulate per token; add the b2 term (folded out of
    # the device kernel). Tokens are unique within an expert, so the
    # fancy-indexed += is safe.
    out = np.einsum("tk,tkd->td", np.asarray(gate_score, dtype=np.float32),
                    b2[np.asarray(gate_idx).astype(np.int64)])
    out = np.ascontiguousarray(out, dtype=np.float32)
    for e in range(E):
        core, le = divmod(e, EPC)
        toks, wts, otoks, owts = routes[e]
        if len(toks):
            # yt [128, MD*C] partition-major -> [D, C]
            ytp = res.results[core]["yt"][le].reshape(128, MD, C)
            y = ytp.transpose(1, 0, 2).reshape(D, C)[:, :len(toks)]
            out[toks] += wts[:, None] * y.T.astype(np.float32)
        if len(otoks):  # exact host fallback for capacity overflow
            hh = inp[otoks] @ w1[e].T + b1[e]
            hh = 0.5 * hh * (1.0 + np.tanh(
                np.sqrt(2.0 / np.pi) * (hh + 0.044715 * hh ** 3)))
            out[otoks] += owts[:, None] * (hh @ w2[e].T)
    return out
